# revision 41
# baseline (speedup 1.0000x reference)
"""Multi-head self-attention (B=4, T=2048, C=1024, 16 heads x hd=64) on 8
Trainium2 NeuronCores.

Sharding: tensor-parallel over heads — each core owns 2 heads (128 of the
1024 channels): its slices of Wq/Wk/Wv rows and Wo columns. Every core reads
the full x (transposed + bf16-cast on host), computes Q^T/K^T (channel-major)
and V (token-major) for its heads, runs attention entirely from SBUF, then
produces a rank-128 partial of the output projection. The 8 partials are
summed on host (+ bo).

Per-core dataflow (all matmuls bf16 in / fp32 PSUM accumulate):
  phase 1: Q^T = Wq_c @ x^T (+bq), K^T = Wk_c @ x^T (bk dropped — it only
           shifts every score in a softmax row by a constant), V = x @ Wv_c^T
           token-major with a ones column appended per head (denominator
           trick) and bv folded into V (softmax weights sum to 1, so adding
           bv to every V row adds exactly bv to the output).
  phase 2: a FLAT software pipeline over steps t = (g, jp) (g = global
           512-query block, jp = 256-key pair): S^T(t) [128k, 1024] per head
           via K^T-stationary matmuls (contraction d=64), one exp per k-tile
           pair on ScalarE (scale=1/8 folded in) -> P^T bf16, and PV(t-2):
           O^T[65,512] += [V|1]^T P^T. The S stream runs two steps ahead of
           the PV stream ACROSS q-block boundaries, so the ScalarE exp
           backlog never gates the next block's S matmuls (st PSUM bufs=2
           recycling) and the PE never drains at block seams.
           Normalize (after PV(g,7)): VectorE copies pv->m (releasing the pv
           PSUM banks), the denominator row is DMA-reshaped to [128,4] so
           the reciprocal runs 128 DVE lanes wide, DMA'd back to [1,512]
           bf16, broadcast over 64 partitions with a K=1 ones matmul on the
           PE, then ot = m * bps. The broadcast+mul are deferred into the
           filler stream so the in-order PE never waits on the DMA
           round-trip (ot is only read 4 q-blocks later); the last q-block
           instead computes 1/denom = exp(-ln denom) on the then-idle
           ScalarE (Ln and Exp share an activation table).
  phase 3: partial_out[128 rows, 1024] = O^T-slice-stationary matmuls against
           Wo_c^T; fp16 partials DMA'd out alternating between the Sync
           hwdge queue and the GpSimd DGE queue; the last 4 tiles split each
           row-tile across BOTH queues and their PSUM->SBUF casts across
           ScalarE+VectorE so the drain is not single-queue bound.

Scheduling: a global FIFO of small (~0.5-2us) filler closures is drained at
3 slots per pipeline step (after S h0's exp, after the S group, after the PV
group). Filler supply is spread across each block's jps (Q proj for g+1 at
jp0, x loads at jp1/jp5, K/V projections for g+4 at jp2-4, output-projection
tiles of g-4 at jp6-7; during the last batch also g-1's tiles) so the FIFO
never runs dry at block seams and keeps ~2 tiles in reserve for the final
normalize window. x and weight loads are single-dispatch DMAs ordered so
the first projection matmuls only wait on half of wk + half of x.

Run-to-run HW time varies ~±15us with the device's power-throttle state
(throttle_active_nc0_time_ns in the profile); compare configs on
exec_time - 0.5*throttle_active.
"""
import json

import numpy as np
import ml_dtypes

import concourse.bass as bass
import concourse.mybir as mybir
import concourse.tile as tile
from concourse.bass_utils import run_bass_kernel_spmd

bf16 = ml_dtypes.bfloat16
f8e4 = ml_dtypes.float8_e4m3fn
dt = mybir.dt

EMB = 1024
HEADS = 16
HD = 64
B = 4
T = 2048
R = B * T            # 8192 rows
NCORES = 8
F = EMB // NCORES    # 128 channels (2 heads) per core
NH = F // HD         # 2 heads per core
NKC = EMB // 128     # 8 contraction chunks for projections
NQB = T // 512       # 4 query blocks per batch
NJP = T // 256       # 8 k-tile PAIRS per batch
NG = R // 512        # 16 global query blocks
G = R // 128         # 64 global row/key tiles
VW = HD + 1          # 65: V head slice + ones column
NSTEP = NG * NJP     # 128 pipeline steps


# ---------------------------------------------------------------------------
# walrus in this container accepts only ONE sync-wait per instruction; split
# extra waits onto same-engine NoOps at BIR-serialization time.
_orig_to_json_bytes = bass.Bass.to_json_bytes


def _split_waits(data: bytes) -> bytes:
    d = json.loads(data)
    changed = False
    for f in d.get("functions", []):
        for blk in f.get("blocks", []):
            out = []
            for inst in blk.get("instructions", []):
                si = inst.get("sync_info")
                waits = (si or {}).get("on_wait") or []
                if len(waits) > 1:
                    changed = True
                    for i, w in enumerate(waits[:-1]):
                        out.append({
                            "debug": inst.get("debug", 0),
                            "engine": inst["engine"],
                            "ins": [], "outs": [],
                            "name": f"{inst['name']}_w{i}",
                            "opcode": "NoOp",
                            "sync_info": {"on_update": [], "on_wait": [w]},
                            "text_hint": "wait_split",
                        })
                    si["on_wait"] = waits[-1:]
                out.append(inst)
            blk["instructions"] = out
    return json.dumps(d).encode() if changed else data


def _to_json_bytes(self, *a, **k):
    return _split_waits(_orig_to_json_bytes(self, *a, **k))


bass.Bass.to_json_bytes = _to_json_bytes
# ---------------------------------------------------------------------------


def build_bass() -> bass.Bass:
    nc = bass.Bass()
    xt_ext = nc.declare_dram_parameter("xt", [EMB, R], dt.bfloat16, isOutput=False)
    xt8_ext = nc.declare_dram_parameter("xt8", [EMB, R], dt.float8e4, isOutput=False)
    wq8_ext = nc.declare_dram_parameter("wq8", [128, 8 * F], dt.float8e4, isOutput=False)
    wk8_ext = nc.declare_dram_parameter("wk8", [128, 8 * F], dt.float8e4, isOutput=False)
    wv_ext = nc.declare_dram_parameter("wv", [EMB, F], dt.bfloat16, isOutput=False)
    wo_ext = nc.declare_dram_parameter("wo", [F, EMB], dt.bfloat16, isOutput=False)
    bq_ext = nc.declare_dram_parameter("bq", [F, 1], dt.float32, isOutput=False)
    bv_ext = nc.declare_dram_parameter("bv", [1, F], dt.float32, isOutput=False)
    out_ext = nc.declare_dram_parameter("out", [R, EMB], dt.float16, isOutput=True)

    Exp = mybir.ActivationFunctionType.Exp
    Ln = mybir.ActivationFunctionType.Ln

    with tile.TileContext(nc) as tc:
        with (
            tc.tile_pool(name="const", bufs=1) as cp,
            tc.tile_pool(name="res", bufs=1) as res,
            tc.tile_pool(name="xt", bufs=1) as xp,
            tc.tile_pool(name="pt", bufs=8) as ptp,
            tc.tile_pool(name="norm", bufs=2) as npl,
            tc.tile_pool(name="osb", bufs=5) as op,
            tc.tile_pool(name="ps", bufs=1, space="PSUM") as ps,
        ):
            # --- constants ---
            # wq8/wk8: host-packed fp8e4 [p, (j, i, m)] with emb row
            # c = j*256 + i*128 + p, weights pre-scaled by 16 (keeps e4m3
            # out of subnormals); the 16*16 factor is folded into the exp
            # scale. DoubleRow halves the Q/K projection PE time.
            wq8_sb = cp.tile([128, 8 * F], dt.float8e4, tag="wq8")
            wk8_sb = cp.tile([128, 8 * F], dt.float8e4, tag="wk8")
            wv_sb = cp.tile([128, EMB], dt.bfloat16, tag="wv")
            wo_sb = cp.tile([128, EMB], dt.bfloat16, tag="wo")
            bq_sb = cp.tile([F, 1], dt.float32, tag="bq")
            bvb_sb = cp.tile([128, F], dt.float32, tag="bvb")
            ones_sb = cp.tile([1, HD], dt.bfloat16, tag="ones")

            def _wload_half(ext, tile_sb, half):
                nc.gpsimd.dma_start(
                    tile_sb[:, half * 4 * F:(half + 1) * 4 * F]
                    .rearrange("p (kc f) -> p kc f", f=F),
                    ext[half * 512:half * 512 + 512, :]
                    .rearrange("(kc p) f -> p kc f", p=128),
                )

            # --- residents ---
            qt_sb = res.tile([F, R], dt.bfloat16, tag="qt")
            kt_sb = res.tile([F, R], dt.bfloat16, tag="kt")
            ot_sb = res.tile([F, R], dt.bfloat16, tag="ot")
            va_sb = res.tile([128, G * NH * VW], dt.bfloat16, tag="va")

            # ---- x loads: two dispatches per 512-row block (the split lets
            # the first projection matmuls start after half the data) ----
            def load_x_half(rb, tag, half, bufs=4):
                xt = xp.tile([128, 4 * 512], dt.bfloat16,
                             tag=f"{tag}{half}", bufs=bufs,
                             name=f"{tag}{half}_{rb}")
                nc.gpsimd.dma_start(
                    xt[:].rearrange("p (kc f) -> p kc f", f=512),
                    xt_ext[half * 512:half * 512 + 512,
                           rb * 512:rb * 512 + 512]
                    .rearrange("(kc p) f -> p kc f", p=128),
                )
                return xt

            def load_x(rb, tag, bufs=4):
                return [load_x_half(rb, tag, h, bufs) for h in range(2)]

            def xsl(xts, kc, lo, hi):
                base = (kc % 4) * 512
                return xts[kc // 4][:, base + lo:base + hi]

            # fp8 x for the Q/K projections: [128 p, (j-pair, i, t)] with
            # emb row c = j*256 + i*128 + p. One 3-D DMA per j (the 4-D
            # combined AP hits ap-balancing bugs in the DMA layer).
            def load_x8_half(rb, tag, half, bufs=4):
                xt = xp.tile([128, 2 * 2 * 512], dt.float8e4,
                             tag=f"{tag}{half}", bufs=bufs,
                             name=f"{tag}{half}_{rb}")
                xtv = xt[:].rearrange("p (j i t) -> p j i t", j=2, i=2)
                for jj in range(2):
                    j = half * 2 + jj
                    nc.sync.dma_start(
                        xtv[:, jj],
                        xt8_ext[j * 256:(j + 1) * 256,
                                rb * 512:rb * 512 + 512]
                        .rearrange("(i p) t -> p i t", i=2),
                    )
                return xt

            def load_x8(rb, tag, bufs=4):
                return [load_x8_half(rb, tag, h, bufs) for h in range(2)]

            def x8sl(x8ts, j):
                return x8ts[j // 2][:].rearrange(
                    "p (j i t) -> p j i t", j=2, i=2)[:, j % 2]

            def w8sl(w8_sb, j):
                return w8_sb[:].rearrange(
                    "p (j i m) -> p j i m", j=4, i=2)[:, j]

            def p1_qk8(rb, x8t, w8_sb, dst_sb, bias, tag="pp"):
                r0 = rb * 512
                acc = ps.tile([128, 512], dt.float32, tag=tag, bufs=2,
                              name=f"prj8_{rb}_{id(w8_sb)}")
                for j in range(4):
                    nc.tensor.matmul(
                        acc[:], w8sl(w8_sb, j), x8sl(x8t, j),
                        start=(j == 0), stop=(j == 3),
                        perf_mode=mybir.MatmulPerfMode.DoubleRow,
                    )
                if bias is not None:
                    nc.vector.tensor_scalar_add(
                        dst_sb[:, r0:r0 + 512], acc[:], bias[:])
                else:
                    nc.vector.tensor_copy(dst_sb[:, r0:r0 + 512], acc[:])

            # ---- projection emitters ----
            def p1_qk(rb, xt, w_sb, dst_sb, bias, tag="pp"):
                r0 = rb * 512
                acc = ps.tile([128, 512], dt.float32, tag=tag, bufs=2,
                              name=f"prj_{rb}_{id(w_sb)}")
                for kc in range(NKC):
                    nc.tensor.matmul(
                        acc[:], w_sb[:, kc * F:(kc + 1) * F],
                        xsl(xt, kc, 0, 512),
                        start=(kc == 0), stop=(kc == NKC - 1),
                    )
                if bias is not None:
                    nc.vector.tensor_scalar_add(
                        dst_sb[:, r0:r0 + 512], acc[:], bias[:])
                else:
                    nc.vector.tensor_copy(dst_sb[:, r0:r0 + 512], acc[:])

            def p1_v(rb, xt, sub):
                g = rb * 4 + sub
                acc = ps.tile([128, F], dt.float32, tag="pp", bufs=2,
                              name=f"vprj_{g}")
                for kc in range(NKC):
                    nc.tensor.matmul(
                        acc[:],
                        xsl(xt, kc, sub * 128, (sub + 1) * 128),
                        wv_sb[:, kc * F:(kc + 1) * F],
                        start=(kc == 0), stop=(kc == NKC - 1),
                    )
                dst = va_sb[:, g * NH * VW:(g + 1) * NH * VW].rearrange(
                    "p (h d) -> p h d", d=VW
                )[:, :, 0:HD]
                nc.vector.tensor_add(
                    dst, acc[:].rearrange("p (h d) -> p h d", d=HD),
                    bvb_sb[:].rearrange("p (h d) -> p h d", d=HD),
                )

            # ---- phase-3 emitter (one 128-row tile); out DMA alternates
            # between the Sync hwdge queue and the GpSimd DGE queue ----
            def p3_tile(g):
                o_sb = op.tile([128, EMB], dt.float16, tag="osb", name=f"o_{g}")
                for ch in range(2):
                    o_ps = ps.tile([128, 512], dt.float32, tag="pp", bufs=2,
                                   name=f"ops_{g}_{ch}")
                    nc.tensor.matmul(
                        o_ps[:],
                        ot_sb[:, g * 128:(g + 1) * 128],
                        wo_sb[:, ch * 512:(ch + 1) * 512],
                        start=True, stop=True,
                    )
                    nc.vector.tensor_copy(o_sb[:, ch * 512:(ch + 1) * 512], o_ps[:])
                nc.gpsimd.dma_start(out_ext[g * 128:(g + 1) * 128, :], o_sb[:])

            # last 4 tiles: casts split across ScalarE+VectorE, out DMA split
            # across both queues so the final drain is not single-queue bound
            def p3_tile_tail(g):
                o_sb = op.tile([128, EMB], dt.float16, tag="osb", name=f"o_{g}")
                for ch in range(2):
                    o_ps = ps.tile([128, 512], dt.float32, tag="pp", bufs=2,
                                   name=f"ops_{g}_{ch}")
                    nc.tensor.matmul(
                        o_ps[:],
                        ot_sb[:, g * 128:(g + 1) * 128],
                        wo_sb[:, ch * 512:(ch + 1) * 512],
                        start=True, stop=True,
                    )
                    if ch == 0:
                        nc.scalar.copy(o_sb[:, 0:512], o_ps[:])
                        nc.gpsimd.dma_start(
                            out_ext[g * 128:(g + 1) * 128, 0:512],
                            o_sb[:, 0:512])
                    else:
                        nc.vector.tensor_copy(o_sb[:, 512:1024], o_ps[:])
                        nc.sync.dma_start(
                            out_ext[g * 128:(g + 1) * 128, 512:1024],
                            o_sb[:, 512:1024])

            # ---- global filler FIFO ----
            fifo = []

            def fill(n=1):
                for _ in range(n):
                    if fifo:
                        fifo.pop(0)()

            # ---- flat-pipeline emitters ----
            pts = {}
            pvs_by_g = {}

            def emit_st(g, jp):
                b, qb = g // NQB, g % NQB
                q0 = b * T + qb * 512
                k0 = b * T + jp * 256
                for h in range(NH):
                    st = ps.tile([128, 1024], dt.float32, tag="st", bufs=2,
                                 name=f"st_{g}_{jp}_{h}")
                    for half in range(2):
                        nc.tensor.matmul(
                            st[:, half * 512:(half + 1) * 512],
                            kt_sb[h * HD:(h + 1) * HD,
                                  k0 + half * 128:k0 + (half + 1) * 128],
                            qt_sb[h * HD:(h + 1) * HD, q0:q0 + 512],
                            start=True, stop=True,
                        )
                    pt = ptp.tile([128, 1024], dt.bfloat16, tag="pt",
                                  name=f"pt_{g}_{jp}_{h}")
                    nc.scalar.activation(pt[:], st[:], Exp,
                                         scale=0.125 / 256.0)
                    pts[(g, jp, h)] = pt

            def emit_pv(g, jp):
                b = g // NQB
                if jp == 0:
                    pvs_by_g[g] = {
                        h: ps.tile([VW, 512], dt.float32, tag="pv", bufs=2,
                                   name=f"pv_{g}_{h}")
                        for h in range(NH)
                    }
                pvs = pvs_by_g[g]
                g0 = b * NJP * 2 + jp * 2
                for h in range(NH):
                    pt = pts.pop((g, jp, h))
                    for half in range(2):
                        gi = g0 + half
                        va = va_sb[:, gi * NH * VW + h * VW:
                                   gi * NH * VW + (h + 1) * VW]
                        nc.tensor.matmul(
                            pvs[h][:], va[:],
                            pt[:, half * 512:(half + 1) * 512],
                            start=(jp == 0 and half == 0),
                            stop=(jp == NJP - 1 and half == 1),
                        )

            # ---- normalize after PV(g, 7) ----
            def normalize(g):
                b, qb = g // NQB, g % NQB
                q0 = b * T + qb * 512
                pvs = pvs_by_g.pop(g)
                # bps borrows the st ring (its previous occupant's exp is
                # long done) so interleaved p3 tiles can cycle the pp ring
                # without deadlocking the in-order PE.
                bps_full = ps.tile([128, 1024], dt.float32, tag="st",
                                   bufs=2, name=f"bps_{g}")
                bps = bps_full[:, 0:512]
                ms = []
                d4 = npl.tile([128, 2 * 4], dt.float32, tag="d4",
                              name=f"d4_{g}")
                for h in range(NH):
                    m = npl.tile([VW, 512], dt.float32, tag="m", bufs=4,
                                 name=f"m_{g}_{h}")
                    nc.vector.tensor_copy(m[:], pvs[h][:])
                    ms.append(m)
                    nc.sync.dma_start(
                        d4[:, h * 4:(h + 1) * 4]
                        .rearrange("p (a c) -> p a c", c=4),
                        m[HD:VW, :].rearrange("p (a c) -> p a c", c=4),
                    )
                r4 = npl.tile([128, 2 * 4], dt.float32, tag="r4",
                              name=f"r4_{g}")
                nc.vector.reciprocal(r4[:], d4[:])
                r4b = npl.tile([128, 2 * 4], dt.bfloat16, tag="r4b",
                               name=f"r4b_{g}")
                with nc.allow_low_precision(reason="1/denom bf16: 2^-9 rel"):
                    nc.vector.tensor_copy(r4b[:], r4[:])
                rcbs = []
                for h in range(NH):
                    rcb = npl.tile([1, 512], dt.bfloat16, tag="rcb", bufs=4,
                                   name=f"rcb_{g}_{h}")
                    nc.sync.dma_start(
                        rcb[:].rearrange("p (a c) -> p a c", c=4),
                        r4b[:, h * 4:(h + 1) * 4],
                    )
                    rcbs.append(rcb)

                def finish():
                    # emitted into the filler stream so the in-order PE
                    # never waits on the rcb DMA round-trip
                    bps = ps.tile([128, 512], dt.float32, tag="pp", bufs=2,
                                  name=f"bps_{g}")
                    for h in range(NH):
                        nc.tensor.matmul(
                            bps[h * HD:(h + 1) * HD, :], ones_sb[:],
                            rcbs[h][:], start=True, stop=True,
                        )
                    for h in range(NH):
                        nc.vector.tensor_mul(
                            ot_sb[h * HD:(h + 1) * HD, q0:q0 + 512],
                            ms[h][0:HD, :], bps[h * HD:(h + 1) * HD, :],
                        )
                # append at the END: the in-order PE must not reach the
                # broadcast matmuls before the rcb DMA round-trip (~2.5us)
                # completes, or it head-of-line blocks the whole stream.
                fifo.append(finish)

            def normalize_last(g):
                # tail: 1/denom = exp(-ln denom) on the (now idle) ScalarE,
                # both heads' Ln/Exp chains uninterrupted, then the reserved
                # p3 tiles (46,47) and the final block's own tiles with
                # split-engine casts and dual-queue DMAs.
                b, qb = g // NQB, g % NQB
                q0 = b * T + qb * 512
                pvs = pvs_by_g.pop(g)
                # bps borrows the st ring (its previous occupant's exp is
                # long done) so interleaved p3 tiles can cycle the pp ring
                # without deadlocking the in-order PE.
                bps_full = ps.tile([128, 1024], dt.float32, tag="st",
                                   bufs=2, name=f"bps_{g}")
                bps = bps_full[:, 0:512]
                ms = []
                for h in range(NH):
                    m = npl.tile([VW, 512], dt.float32, tag="m", bufs=4,
                                 name=f"m_{g}_{h}")
                    nc.vector.tensor_copy(m[:], pvs[h][:])
                    ms.append(m)
                rcbs = []
                for h in range(NH):
                    ld = npl.tile([1, 512], dt.float32, tag="ld",
                                  name=f"ld_{g}_{h}")
                    nc.scalar.activation(ld[:], ms[h][HD:VW, :], Ln)
                    rcb = npl.tile([1, 512], dt.bfloat16, tag="rcb",
                                   bufs=4, name=f"rcbL_{g}_{h}")
                    nc.scalar.activation(rcb[:], ld[:], Exp, scale=-1.0)
                    rcbs.append(rcb)
                for h in range(NH):
                    nc.tensor.matmul(
                        bps[h * HD:(h + 1) * HD, :], ones_sb[:], rcbs[h][:],
                        start=True, stop=True,
                    )
                for h in range(NH):
                    nc.vector.tensor_mul(
                        ot_sb[h * HD:(h + 1) * HD, q0:q0 + 512],
                        ms[h][0:HD, :], bps[h * HD:(h + 1) * HD, :],
                    )
                p3_tile_tail(46)
                p3_tile_tail(47)

            # ---------------- emission schedule ----------------
            # DMA dispatch order = consumption order: the first projection
            # chains (K then Q for row-block 0) only need wk/wq half0 +
            # x half0 for their first four contraction chunks.
            nc.sync.dma_start(wk8_sb[:], wk8_ext[:])
            x8kv = {0: load_x8(0, "x8kv")}
            nc.sync.dma_start(wq8_sb[:], wq8_ext[:])
            nc.sync.dma_start(bq_sb[:], bq_ext[:])
            x8q = {1: load_x8(1, "x8q")}
            _wload_half(wv_ext, wv_sb, 0)
            _wload_half(wv_ext, wv_sb, 1)
            xkv = {0: load_x(0, "xtkv")}
            nc.gpsimd.dma_start(
                bvb_sb[:],
                bv_ext[:].rearrange("o (p f) -> o p f", p=1)
                .broadcast_to((1, 128, F)),
            )
            for rb in range(1, 4):
                x8kv[rb] = load_x8(rb, "x8kv")
                xkv[rb] = load_x(rb, "xtkv")
            nc.gpsimd.dma_start(wo_sb[:], wo_ext[:])
            nc.vector.memset(ones_sb[:], 1.0)
            nc.vector.memset(
                va_sb[:].rearrange("p (g d) -> p g d", d=VW)[:, :, HD:VW], 1.0
            )

            # upfront: K/Q projections for row-block 0 only; V(0) and K/V for
            # row-blocks 1-3 ride the filler FIFO inside the first q-block,
            # ordered by their S/PV consumption deadlines.
            p1_qk8(0, x8kv[0], wk8_sb, kt_sb, None)
            p1_qk8(0, x8kv[0], wq8_sb, qt_sb, bq_sb)
            fifo.append(lambda: p1_qk8(1, x8q[1], wq8_sb, qt_sb, bq_sb))
            # FIFO ordered by S/PV consumption deadline at 2-3 pulls/step:
            # K(rb) before S(0, 2rb) at t=2rb; V(rb) before PV(0, 2rb) at
            # t=2rb+2.
            fifo.append(lambda: p1_v(0, xkv[0], 0))
            fifo.append(lambda: p1_v(0, xkv[0], 1))
            fifo.append(lambda rb=1: p1_qk8(rb, x8kv[rb], wk8_sb, kt_sb, None))
            fifo.append(lambda: p1_v(0, xkv[0], 2))
            fifo.append(lambda: p1_v(0, xkv[0], 3))
            fifo.append(lambda rb=2: p1_qk8(rb, x8kv[rb], wk8_sb, kt_sb, None))
            fifo.append(lambda: p1_v(1, xkv[1], 0))
            fifo.append(lambda: p1_v(1, xkv[1], 1))
            fifo.append(lambda rb=3: p1_qk8(rb, x8kv[rb], wk8_sb, kt_sb, None))
            fifo.append(lambda: p1_v(1, xkv[1], 2))
            fifo.append(lambda: p1_v(1, xkv[1], 3))
            for rb in range(2, 4):
                for sub in range(4):
                    fifo.append(lambda rb=rb, s=sub: p1_v(rb, xkv[rb], s))
            x8kv[4] = load_x8(4, "x8kv")
            xkv[4] = load_x(4, "xtkv")

            # per-(g, jp) filler supply, spread across the block so the FIFO
            # never runs dry at block seams
            def enqueue(g, jp):
                if jp == 0 and 0 < g and g + 1 < NG:
                    # (Q(1) is enqueued upfront, fed by the early x8q[1])
                    fifo.append(
                        lambda rb=g + 1: p1_qk8(rb, x8q[rb], wq8_sb, qt_sb,
                                                bq_sb))
                elif jp == 1 and g + 2 < NG:
                    def lq(rb=g + 2):
                        x8q[rb] = load_x8(rb, "x8q")
                    fifo.append(lq)
                elif jp == 2 and g + 4 < NG:
                    fifo.append(
                        lambda rb=g + 4: p1_qk8(rb, x8kv[rb], wk8_sb, kt_sb,
                                                None))
                elif jp == 3 and g + 4 < NG:
                    fifo.append(lambda rb=g + 4: p1_v(rb, xkv[rb], 0))
                    fifo.append(lambda rb=g + 4: p1_v(rb, xkv[rb], 1))
                elif jp == 4 and g + 4 < NG:
                    fifo.append(lambda rb=g + 4: p1_v(rb, xkv[rb], 2))
                    fifo.append(lambda rb=g + 4: p1_v(rb, xkv[rb], 3))
                elif jp == 5 and g + 5 < NG:
                    def lkv(rb=g + 5):
                        x8kv[rb] = load_x8(rb, "x8kv")
                        xkv[rb] = load_x(rb, "xtkv")
                    fifo.append(lkv)
                elif jp == 4 and g == 15:
                    fifo.append(lambda: p3_tile(44))
                elif jp == 5 and g == 15:
                    # 44/45 early enough that their VectorE casts drain
                    # before the final normalize's m copies; 46/47 are
                    # reserved for normalize_last itself
                    fifo.append(lambda: p3_tile(45))
                elif jp == 6 and g >= 4 and g != 15:
                    for tt in range(4 * (g - 4), 4 * (g - 4) + 2):
                        fifo.append(lambda tt=tt: p3_tile(tt))
                elif jp == 7 and g >= 4 and g != 15:
                    for tt in range(4 * (g - 4) + 2, 4 * (g - 4) + 4):
                        fifo.append(lambda tt=tt: p3_tile(tt))
                if g >= 13 and 3 <= jp <= 6:
                    # last batch: also drip the previous block's p3 tiles
                    tt = 4 * (g - 1) + (jp - 3)
                    fifo.append(lambda tt=tt: p3_tile(tt))

            # ---- the flat pipeline. One fill point per step (chunky chains
            # minimize PE semaphore-check hops, ~90ns per chain switch) ----
            for t in range(NSTEP + 2):
                if t < NSTEP:
                    g, jp = divmod(t, NJP)
                    enqueue(g, jp)
                    emit_st(g, jp)
                if t >= 2:
                    pg, pjp = divmod(t - 2, NJP)
                    emit_pv(pg, pjp)
                fill(3 if t < 4 else 2)
                if t >= 2:
                    pg, pjp = divmod(t - 2, NJP)
                    if pjp == NJP - 1:
                        if pg == NG - 1:
                            normalize_last(pg)
                        else:
                            normalize(pg)
            # tail: drain leftovers + output projection of the last q-block
            fill(len(fifo))
            for tt in range(60, 64):
                p3_tile_tail(tt)
    return nc


_NC_CACHE = None


def _get_nc():
    global _NC_CACHE
    if _NC_CACHE is None:
        _NC_CACHE = build_bass()
    return _NC_CACHE


def make_in_maps(x, Wq, bq, Wk, bk, Wv, bv, Wo, bo):
    xt = np.ascontiguousarray(
        np.asarray(x, dtype=np.float32).reshape(R, EMB).astype(bf16).T
    )
    xt8 = np.ascontiguousarray(
        np.asarray(x, dtype=np.float32).reshape(R, EMB).astype(f8e4).T
    )

    def pack_w8(W, rows):
        # [p, (j, i, m)] with emb row c = j*256 + i*128 + p; 16x pre-scale
        Wc = np.ascontiguousarray((np.asarray(W)[rows, :] * 16.0).T)  # [c, m]
        arr = Wc.reshape(4, 2, 128, F).transpose(2, 0, 1, 3)  # [p, j, i, m]
        return np.ascontiguousarray(arr.reshape(128, 8 * F).astype(f8e4))
    in_maps = []
    for c in range(NCORES):
        rows = slice(F * c, F * (c + 1))
        in_maps.append({
            "xt": xt,
            "xt8": xt8,
            "wq8": pack_w8(Wq, rows),
            "wk8": pack_w8(Wk, rows),
            "wv": np.ascontiguousarray(np.asarray(Wv)[rows, :].T.astype(bf16)),
            "wo": np.ascontiguousarray(np.asarray(Wo)[:, rows].T.astype(bf16)),
            "bq": (np.asarray(bq)[rows] * 16.0).reshape(F, 1)
            .astype(np.float32),
            "bv": np.asarray(bv)[rows].reshape(1, F).astype(np.float32),
        })
    return in_maps


def gather(results, bo):
    acc = np.zeros((R, EMB), np.float32)
    for r in results:
        acc += r["out"].astype(np.float32)
    acc += np.asarray(bo, dtype=np.float32)
    return acc.reshape(B, T, EMB)


def kernel(x, Wq, bq, Wk, bk, Wv, bv, Wo, bo, _trace=False):
    nc = _get_nc()
    in_maps = make_in_maps(x, Wq, bq, Wk, bk, Wv, bv, Wo, bo)
    res = run_bass_kernel_spmd(nc, in_maps, list(range(NCORES)), trace=_trace)
    out = gather(res.results, bo)
    if _trace:
        kernel.last_result = res
    return out


# revision 42
# speedup vs baseline: 1.0605x; 1.0605x over previous
"""Multi-head self-attention (B=4, T=2048, C=1024, 16 heads x hd=64) on 8
Trainium2 NeuronCores.

Sharding: tensor-parallel over heads — each core owns 2 heads (128 of the
1024 channels): its slices of Wq/Wk/Wv rows and Wo columns. Every core reads
the full x (transposed + bf16-cast on host), computes Q^T/K^T (channel-major)
and V (token-major) for its heads, runs attention entirely from SBUF, then
produces a rank-128 partial of the output projection. The 8 partials are
summed on host (+ bo).

Per-core dataflow (all matmuls bf16 in / fp32 PSUM accumulate):
  phase 1: Q^T = Wq_c @ x^T (+bq), K^T = Wk_c @ x^T (bk dropped — it only
           shifts every score in a softmax row by a constant), V = x @ Wv_c^T
           token-major with a ones column appended per head (denominator
           trick) and bv folded into V (softmax weights sum to 1, so adding
           bv to every V row adds exactly bv to the output).
  phase 2: a FLAT software pipeline over steps t = (g, jp) (g = global
           512-query block, jp = 256-key pair): S^T(t) [128k, 1024] per head
           via K^T-stationary matmuls (contraction d=64), one exp per k-tile
           pair on ScalarE (scale=1/8 folded in) -> P^T bf16, and PV(t-2):
           O^T[65,512] += [V|1]^T P^T. The S stream runs two steps ahead of
           the PV stream ACROSS q-block boundaries, so the ScalarE exp
           backlog never gates the next block's S matmuls (st PSUM bufs=2
           recycling) and the PE never drains at block seams.
           Normalize (after PV(g,7)): VectorE copies pv->m (releasing the pv
           PSUM banks), the denominator row is DMA-reshaped to [128,4] so
           the reciprocal runs 128 DVE lanes wide, DMA'd back to [1,512]
           bf16, broadcast over 64 partitions with a K=1 ones matmul on the
           PE, then ot = m * bps. The broadcast+mul are deferred into the
           filler stream so the in-order PE never waits on the DMA
           round-trip (ot is only read 4 q-blocks later); the last q-block
           instead computes 1/denom = exp(-ln denom) on the then-idle
           ScalarE (Ln and Exp share an activation table).
  phase 3: partial_out[128 rows, 1024] = O^T-slice-stationary matmuls against
           Wo_c^T; fp16 partials DMA'd out alternating between the Sync
           hwdge queue and the GpSimd DGE queue; the last 4 tiles split each
           row-tile across BOTH queues and their PSUM->SBUF casts across
           ScalarE+VectorE so the drain is not single-queue bound.

Scheduling: a global FIFO of small (~0.5-2us) filler closures is drained at
3 slots per pipeline step (after S h0's exp, after the S group, after the PV
group). Filler supply is spread across each block's jps (Q proj for g+1 at
jp0, x loads at jp1/jp5, K/V projections for g+4 at jp2-4, output-projection
tiles of g-4 at jp6-7; during the last batch also g-1's tiles) so the FIFO
never runs dry at block seams and keeps ~2 tiles in reserve for the final
normalize window. x and weight loads are single-dispatch DMAs ordered so
the first projection matmuls only wait on half of wk + half of x.

Run-to-run HW time varies ~±15us with the device's power-throttle state
(throttle_active_nc0_time_ns in the profile); compare configs on
exec_time - 0.5*throttle_active.
"""
import json

import numpy as np
import ml_dtypes

import concourse.bass as bass
import concourse.mybir as mybir
import concourse.tile as tile
from concourse.bass_utils import run_bass_kernel_spmd

bf16 = ml_dtypes.bfloat16
f8e4 = ml_dtypes.float8_e4m3fn
dt = mybir.dt

EMB = 1024
HEADS = 16
HD = 64
B = 4
T = 2048
R = B * T            # 8192 rows
NCORES = 8
F = EMB // NCORES    # 128 channels (2 heads) per core
NH = F // HD         # 2 heads per core
NKC = EMB // 128     # 8 contraction chunks for projections
NQB = T // 512       # 4 query blocks per batch
NJP = T // 256       # 8 k-tile PAIRS per batch
NG = R // 512        # 16 global query blocks
G = R // 128         # 64 global row/key tiles
VW = HD + 1          # 65: V head slice + ones column
NSTEP = NG * NJP     # 128 pipeline steps


# ---------------------------------------------------------------------------
# walrus in this container accepts only ONE sync-wait per instruction; split
# extra waits onto same-engine NoOps at BIR-serialization time.
_orig_to_json_bytes = bass.Bass.to_json_bytes


def _split_waits(data: bytes) -> bytes:
    d = json.loads(data)
    changed = False
    for f in d.get("functions", []):
        for blk in f.get("blocks", []):
            out = []
            for inst in blk.get("instructions", []):
                si = inst.get("sync_info")
                waits = (si or {}).get("on_wait") or []
                if len(waits) > 1:
                    changed = True
                    for i, w in enumerate(waits[:-1]):
                        out.append({
                            "debug": inst.get("debug", 0),
                            "engine": inst["engine"],
                            "ins": [], "outs": [],
                            "name": f"{inst['name']}_w{i}",
                            "opcode": "NoOp",
                            "sync_info": {"on_update": [], "on_wait": [w]},
                            "text_hint": "wait_split",
                        })
                    si["on_wait"] = waits[-1:]
                out.append(inst)
            blk["instructions"] = out
    return json.dumps(d).encode() if changed else data


def _to_json_bytes(self, *a, **k):
    return _split_waits(_orig_to_json_bytes(self, *a, **k))


bass.Bass.to_json_bytes = _to_json_bytes
# ---------------------------------------------------------------------------


def build_bass() -> bass.Bass:
    nc = bass.Bass()
    xt_ext = nc.declare_dram_parameter("xt", [EMB, R], dt.bfloat16, isOutput=False)
    xt8_ext = nc.declare_dram_parameter("xt8", [EMB, R], dt.float8e4, isOutput=False)
    wq8_ext = nc.declare_dram_parameter("wq8", [128, 8 * F], dt.float8e4, isOutput=False)
    wk8_ext = nc.declare_dram_parameter("wk8", [128, 8 * F], dt.float8e4, isOutput=False)
    wv_ext = nc.declare_dram_parameter("wv", [EMB, F], dt.bfloat16, isOutput=False)
    wo_ext = nc.declare_dram_parameter("wo", [F, EMB], dt.bfloat16, isOutput=False)
    bq_ext = nc.declare_dram_parameter("bq", [F, 1], dt.float32, isOutput=False)
    bv_ext = nc.declare_dram_parameter("bv", [1, F], dt.float32, isOutput=False)
    out_ext = nc.declare_dram_parameter("out", [R, EMB], dt.float16, isOutput=True)

    Exp = mybir.ActivationFunctionType.Exp
    Ln = mybir.ActivationFunctionType.Ln

    with tile.TileContext(nc) as tc:
        with (
            tc.tile_pool(name="const", bufs=1) as cp,
            tc.tile_pool(name="res", bufs=1) as res,
            tc.tile_pool(name="xt", bufs=1) as xp,
            tc.tile_pool(name="pt", bufs=8) as ptp,
            tc.tile_pool(name="norm", bufs=2) as npl,
            tc.tile_pool(name="osb", bufs=5) as op,
            tc.tile_pool(name="ps", bufs=1, space="PSUM") as ps,
        ):
            # --- constants ---
            # wq8/wk8: host-packed fp8e4 [p, (j, i, m)] with emb row
            # c = j*256 + i*128 + p, weights pre-scaled by 16 (keeps e4m3
            # out of subnormals); the 16*16 factor is folded into the exp
            # scale. DoubleRow halves the Q/K projection PE time.
            wq8_sb = cp.tile([128, 8 * F], dt.float8e4, tag="wq8")
            wk8_sb = cp.tile([128, 8 * F], dt.float8e4, tag="wk8")
            wv_sb = cp.tile([128, EMB], dt.bfloat16, tag="wv")
            wo_sb = cp.tile([128, EMB], dt.bfloat16, tag="wo")
            bq_sb = cp.tile([F, 1], dt.float32, tag="bq")
            bvb_sb = cp.tile([128, F], dt.float32, tag="bvb")
            ones_sb = cp.tile([1, HD], dt.bfloat16, tag="ones")

            def _wload_half(ext, tile_sb, half):
                nc.sync.dma_start(
                    tile_sb[:, half * 4 * F:(half + 1) * 4 * F]
                    .rearrange("p (kc f) -> p kc f", f=F),
                    ext[half * 512:half * 512 + 512, :]
                    .rearrange("(kc p) f -> p kc f", p=128),
                )

            # --- residents ---
            qt_sb = res.tile([F, R], dt.bfloat16, tag="qt")
            kt_sb = res.tile([F, R], dt.bfloat16, tag="kt")
            ot_sb = res.tile([F, R], dt.bfloat16, tag="ot")
            va_sb = res.tile([128, G * NH * VW], dt.bfloat16, tag="va")

            # ---- x loads: two dispatches per 512-row block (the split lets
            # the first projection matmuls start after half the data) ----
            def load_x_half(rb, tag, half, bufs=4):
                xt = xp.tile([128, 4 * 512], dt.bfloat16,
                             tag=f"{tag}{half}", bufs=bufs,
                             name=f"{tag}{half}_{rb}")
                nc.sync.dma_start(
                    xt[:].rearrange("p (kc f) -> p kc f", f=512),
                    xt_ext[half * 512:half * 512 + 512,
                           rb * 512:rb * 512 + 512]
                    .rearrange("(kc p) f -> p kc f", p=128),
                )
                return xt

            def load_x(rb, tag, bufs=4):
                return [load_x_half(rb, tag, h, bufs) for h in range(2)]

            def xsl(xts, kc, lo, hi):
                base = (kc % 4) * 512
                return xts[kc // 4][:, base + lo:base + hi]

            # fp8 x for the Q/K projections: [128 p, (j-pair, i, t)] with
            # emb row c = j*256 + i*128 + p. One 3-D DMA per j (the 4-D
            # combined AP hits ap-balancing bugs in the DMA layer).
            def load_x8_half(rb, tag, half, bufs=4):
                xt = xp.tile([128, 2 * 2 * 512], dt.float8e4,
                             tag=f"{tag}{half}", bufs=bufs,
                             name=f"{tag}{half}_{rb}")
                xtv = xt[:].rearrange("p (j i t) -> p j i t", j=2, i=2)
                for jj in range(2):
                    j = half * 2 + jj
                    nc.sync.dma_start(
                        xtv[:, jj],
                        xt8_ext[j * 256:(j + 1) * 256,
                                rb * 512:rb * 512 + 512]
                        .rearrange("(i p) t -> p i t", i=2),
                    )
                return xt

            def load_x8(rb, tag, bufs=4):
                return [load_x8_half(rb, tag, h, bufs) for h in range(2)]

            def x8sl(x8ts, j):
                return x8ts[j // 2][:].rearrange(
                    "p (j i t) -> p j i t", j=2, i=2)[:, j % 2]

            def w8sl(w8_sb, j):
                return w8_sb[:].rearrange(
                    "p (j i m) -> p j i m", j=4, i=2)[:, j]

            def p1_qk8(rb, x8t, w8_sb, dst_sb, bias, tag="pp"):
                r0 = rb * 512
                acc = ps.tile([128, 512], dt.float32, tag=tag, bufs=2,
                              name=f"prj8_{rb}_{id(w8_sb)}")
                for j in range(4):
                    nc.tensor.matmul(
                        acc[:], w8sl(w8_sb, j), x8sl(x8t, j),
                        start=(j == 0), stop=(j == 3),
                        perf_mode=mybir.MatmulPerfMode.DoubleRow,
                    )
                if bias is not None:
                    nc.vector.tensor_scalar_add(
                        dst_sb[:, r0:r0 + 512], acc[:], bias[:])
                else:
                    nc.vector.tensor_copy(dst_sb[:, r0:r0 + 512], acc[:])

            # ---- projection emitters ----
            def p1_qk(rb, xt, w_sb, dst_sb, bias, tag="pp"):
                r0 = rb * 512
                acc = ps.tile([128, 512], dt.float32, tag=tag, bufs=2,
                              name=f"prj_{rb}_{id(w_sb)}")
                for kc in range(NKC):
                    nc.tensor.matmul(
                        acc[:], w_sb[:, kc * F:(kc + 1) * F],
                        xsl(xt, kc, 0, 512),
                        start=(kc == 0), stop=(kc == NKC - 1),
                    )
                if bias is not None:
                    nc.vector.tensor_scalar_add(
                        dst_sb[:, r0:r0 + 512], acc[:], bias[:])
                else:
                    nc.vector.tensor_copy(dst_sb[:, r0:r0 + 512], acc[:])

            def p1_v(rb, xt, sub):
                g = rb * 4 + sub
                acc = ps.tile([128, F], dt.float32, tag="pp", bufs=2,
                              name=f"vprj_{g}")
                for kc in range(NKC):
                    nc.tensor.matmul(
                        acc[:],
                        xsl(xt, kc, sub * 128, (sub + 1) * 128),
                        wv_sb[:, kc * F:(kc + 1) * F],
                        start=(kc == 0), stop=(kc == NKC - 1),
                    )
                dst = va_sb[:, g * NH * VW:(g + 1) * NH * VW].rearrange(
                    "p (h d) -> p h d", d=VW
                )[:, :, 0:HD]
                nc.vector.tensor_add(
                    dst, acc[:].rearrange("p (h d) -> p h d", d=HD),
                    bvb_sb[:].rearrange("p (h d) -> p h d", d=HD),
                )

            # ---- phase-3 emitter (one 128-row tile); out DMA alternates
            # between the Sync hwdge queue and the GpSimd DGE queue ----
            def p3_tile(g):
                o_sb = op.tile([128, EMB], dt.float16, tag="osb", name=f"o_{g}")
                for ch in range(2):
                    o_ps = ps.tile([128, 512], dt.float32, tag="pp", bufs=2,
                                   name=f"ops_{g}_{ch}")
                    nc.tensor.matmul(
                        o_ps[:],
                        ot_sb[:, g * 128:(g + 1) * 128],
                        wo_sb[:, ch * 512:(ch + 1) * 512],
                        start=True, stop=True,
                    )
                    nc.vector.tensor_copy(o_sb[:, ch * 512:(ch + 1) * 512], o_ps[:])
                nc.gpsimd.dma_start(out_ext[g * 128:(g + 1) * 128, :], o_sb[:])

            # last 4 tiles: casts split across ScalarE+VectorE, out DMA split
            # across both queues so the final drain is not single-queue bound
            def p3_tile_tail(g):
                o_sb = op.tile([128, EMB], dt.float16, tag="osb", name=f"o_{g}")
                for ch in range(2):
                    o_ps = ps.tile([128, 512], dt.float32, tag="pp", bufs=2,
                                   name=f"ops_{g}_{ch}")
                    nc.tensor.matmul(
                        o_ps[:],
                        ot_sb[:, g * 128:(g + 1) * 128],
                        wo_sb[:, ch * 512:(ch + 1) * 512],
                        start=True, stop=True,
                    )
                    if ch == 0:
                        nc.scalar.copy(o_sb[:, 0:512], o_ps[:])
                        nc.gpsimd.dma_start(
                            out_ext[g * 128:(g + 1) * 128, 0:512],
                            o_sb[:, 0:512])
                    else:
                        nc.vector.tensor_copy(o_sb[:, 512:1024], o_ps[:])
                        nc.sync.dma_start(
                            out_ext[g * 128:(g + 1) * 128, 512:1024],
                            o_sb[:, 512:1024])

            # ---- global filler FIFO ----
            fifo = []

            def fill(n=1):
                for _ in range(n):
                    if fifo:
                        fifo.pop(0)()

            # ---- flat-pipeline emitters ----
            pts = {}
            pvs_by_g = {}

            def emit_st(g, jp):
                b, qb = g // NQB, g % NQB
                q0 = b * T + qb * 512
                k0 = b * T + jp * 256
                for h in range(NH):
                    st = ps.tile([128, 1024], dt.float32, tag="st", bufs=2,
                                 name=f"st_{g}_{jp}_{h}")
                    for half in range(2):
                        nc.tensor.matmul(
                            st[:, half * 512:(half + 1) * 512],
                            kt_sb[h * HD:(h + 1) * HD,
                                  k0 + half * 128:k0 + (half + 1) * 128],
                            qt_sb[h * HD:(h + 1) * HD, q0:q0 + 512],
                            start=True, stop=True,
                        )
                    pt = ptp.tile([128, 1024], dt.bfloat16, tag="pt",
                                  name=f"pt_{g}_{jp}_{h}")
                    nc.scalar.activation(pt[:], st[:], Exp,
                                         scale=0.125 / 256.0)
                    pts[(g, jp, h)] = pt

            def emit_pv(g, jp):
                b = g // NQB
                if jp == 0:
                    pvs_by_g[g] = {
                        h: ps.tile([VW, 512], dt.float32, tag="pv", bufs=2,
                                   name=f"pv_{g}_{h}")
                        for h in range(NH)
                    }
                pvs = pvs_by_g[g]
                g0 = b * NJP * 2 + jp * 2
                for h in range(NH):
                    pt = pts.pop((g, jp, h))
                    for half in range(2):
                        gi = g0 + half
                        va = va_sb[:, gi * NH * VW + h * VW:
                                   gi * NH * VW + (h + 1) * VW]
                        nc.tensor.matmul(
                            pvs[h][:], va[:],
                            pt[:, half * 512:(half + 1) * 512],
                            start=(jp == 0 and half == 0),
                            stop=(jp == NJP - 1 and half == 1),
                        )

            # ---- normalize after PV(g, 7) ----
            def normalize(g):
                b, qb = g // NQB, g % NQB
                q0 = b * T + qb * 512
                pvs = pvs_by_g.pop(g)
                # bps borrows the st ring (its previous occupant's exp is
                # long done) so interleaved p3 tiles can cycle the pp ring
                # without deadlocking the in-order PE.
                bps_full = ps.tile([128, 1024], dt.float32, tag="st",
                                   bufs=2, name=f"bps_{g}")
                bps = bps_full[:, 0:512]
                ms = []
                d4 = npl.tile([128, 2 * 4], dt.float32, tag="d4",
                              name=f"d4_{g}")
                for h in range(NH):
                    m = npl.tile([VW, 512], dt.float32, tag="m", bufs=4,
                                 name=f"m_{g}_{h}")
                    nc.vector.tensor_copy(m[:], pvs[h][:])
                    ms.append(m)
                    nc.sync.dma_start(
                        d4[:, h * 4:(h + 1) * 4]
                        .rearrange("p (a c) -> p a c", c=4),
                        m[HD:VW, :].rearrange("p (a c) -> p a c", c=4),
                    )
                r4 = npl.tile([128, 2 * 4], dt.float32, tag="r4",
                              name=f"r4_{g}")
                nc.vector.reciprocal(r4[:], d4[:])
                r4b = npl.tile([128, 2 * 4], dt.bfloat16, tag="r4b",
                               name=f"r4b_{g}")
                with nc.allow_low_precision(reason="1/denom bf16: 2^-9 rel"):
                    nc.vector.tensor_copy(r4b[:], r4[:])
                rcbs = []
                for h in range(NH):
                    rcb = npl.tile([1, 512], dt.bfloat16, tag="rcb", bufs=4,
                                   name=f"rcb_{g}_{h}")
                    nc.sync.dma_start(
                        rcb[:].rearrange("p (a c) -> p a c", c=4),
                        r4b[:, h * 4:(h + 1) * 4],
                    )
                    rcbs.append(rcb)

                def finish():
                    # emitted into the filler stream so the in-order PE
                    # never waits on the rcb DMA round-trip
                    bps = ps.tile([128, 512], dt.float32, tag="pp", bufs=2,
                                  name=f"bps_{g}")
                    for h in range(NH):
                        nc.tensor.matmul(
                            bps[h * HD:(h + 1) * HD, :], ones_sb[:],
                            rcbs[h][:], start=True, stop=True,
                        )
                    for h in range(NH):
                        nc.vector.tensor_mul(
                            ot_sb[h * HD:(h + 1) * HD, q0:q0 + 512],
                            ms[h][0:HD, :], bps[h * HD:(h + 1) * HD, :],
                        )
                # append at the END: the in-order PE must not reach the
                # broadcast matmuls before the rcb DMA round-trip (~2.5us)
                # completes, or it head-of-line blocks the whole stream.
                fifo.append(finish)

            def normalize_last(g):
                # tail: 1/denom = exp(-ln denom) on the (now idle) ScalarE,
                # both heads' Ln/Exp chains uninterrupted, then the reserved
                # p3 tiles (46,47) and the final block's own tiles with
                # split-engine casts and dual-queue DMAs.
                b, qb = g // NQB, g % NQB
                q0 = b * T + qb * 512
                pvs = pvs_by_g.pop(g)
                # bps borrows the st ring (its previous occupant's exp is
                # long done) so interleaved p3 tiles can cycle the pp ring
                # without deadlocking the in-order PE.
                bps_full = ps.tile([128, 1024], dt.float32, tag="st",
                                   bufs=2, name=f"bps_{g}")
                bps = bps_full[:, 0:512]
                ms = []
                for h in range(NH):
                    m = npl.tile([VW, 512], dt.float32, tag="m", bufs=4,
                                 name=f"m_{g}_{h}")
                    nc.vector.tensor_copy(m[:], pvs[h][:])
                    ms.append(m)
                rcbs = []
                for h in range(NH):
                    ld = npl.tile([1, 512], dt.float32, tag="ld",
                                  name=f"ld_{g}_{h}")
                    nc.scalar.activation(ld[:], ms[h][HD:VW, :], Ln)
                    rcb = npl.tile([1, 512], dt.bfloat16, tag="rcb",
                                   bufs=4, name=f"rcbL_{g}_{h}")
                    nc.scalar.activation(rcb[:], ld[:], Exp, scale=-1.0)
                    rcbs.append(rcb)
                for h in range(NH):
                    nc.tensor.matmul(
                        bps[h * HD:(h + 1) * HD, :], ones_sb[:], rcbs[h][:],
                        start=True, stop=True,
                    )
                for h in range(NH):
                    nc.vector.tensor_mul(
                        ot_sb[h * HD:(h + 1) * HD, q0:q0 + 512],
                        ms[h][0:HD, :], bps[h * HD:(h + 1) * HD, :],
                    )
                p3_tile_tail(46)
                p3_tile_tail(47)

            # ---------------- emission schedule ----------------
            # DMA dispatch order = consumption order: the first projection
            # chains (K then Q for row-block 0) only need wk/wq half0 +
            # x half0 for their first four contraction chunks.
            nc.sync.dma_start(wk8_sb[:], wk8_ext[:])
            x8kv = {0: load_x8(0, "x8kv")}
            nc.sync.dma_start(wq8_sb[:], wq8_ext[:])
            nc.sync.dma_start(bq_sb[:], bq_ext[:])
            x8q = {1: load_x8(1, "x8q")}
            _wload_half(wv_ext, wv_sb, 0)
            _wload_half(wv_ext, wv_sb, 1)
            xkv = {0: load_x(0, "xtkv")}
            nc.sync.dma_start(
                bvb_sb[:],
                bv_ext[:].rearrange("o (p f) -> o p f", p=1)
                .broadcast_to((1, 128, F)),
            )
            for rb in range(1, 4):
                x8kv[rb] = load_x8(rb, "x8kv")
                xkv[rb] = load_x(rb, "xtkv")
            nc.sync.dma_start(wo_sb[:], wo_ext[:])
            nc.vector.memset(ones_sb[:], 1.0)
            nc.vector.memset(
                va_sb[:].rearrange("p (g d) -> p g d", d=VW)[:, :, HD:VW], 1.0
            )

            # upfront: K/Q projections for row-block 0 only; V(0) and K/V for
            # row-blocks 1-3 ride the filler FIFO inside the first q-block,
            # ordered by their S/PV consumption deadlines.
            p1_qk8(0, x8kv[0], wk8_sb, kt_sb, None)
            p1_qk8(0, x8kv[0], wq8_sb, qt_sb, bq_sb)
            fifo.append(lambda: p1_qk8(1, x8q[1], wq8_sb, qt_sb, bq_sb))
            # FIFO ordered by S/PV consumption deadline at 2-3 pulls/step:
            # K(rb) before S(0, 2rb) at t=2rb; V(rb) before PV(0, 2rb) at
            # t=2rb+2.
            fifo.append(lambda: p1_v(0, xkv[0], 0))
            fifo.append(lambda: p1_v(0, xkv[0], 1))
            fifo.append(lambda rb=1: p1_qk8(rb, x8kv[rb], wk8_sb, kt_sb, None))
            fifo.append(lambda: p1_v(0, xkv[0], 2))
            fifo.append(lambda: p1_v(0, xkv[0], 3))
            fifo.append(lambda rb=2: p1_qk8(rb, x8kv[rb], wk8_sb, kt_sb, None))
            fifo.append(lambda: p1_v(1, xkv[1], 0))
            fifo.append(lambda: p1_v(1, xkv[1], 1))
            fifo.append(lambda rb=3: p1_qk8(rb, x8kv[rb], wk8_sb, kt_sb, None))
            fifo.append(lambda: p1_v(1, xkv[1], 2))
            fifo.append(lambda: p1_v(1, xkv[1], 3))
            for rb in range(2, 4):
                for sub in range(4):
                    fifo.append(lambda rb=rb, s=sub: p1_v(rb, xkv[rb], s))
            x8kv[4] = load_x8(4, "x8kv")
            xkv[4] = load_x(4, "xtkv")

            # per-(g, jp) filler supply, spread across the block so the FIFO
            # never runs dry at block seams
            def enqueue(g, jp):
                if jp == 0 and 0 < g and g + 1 < NG:
                    # (Q(1) is enqueued upfront, fed by the early x8q[1])
                    fifo.append(
                        lambda rb=g + 1: p1_qk8(rb, x8q[rb], wq8_sb, qt_sb,
                                                bq_sb))
                elif jp == 1 and g + 2 < NG:
                    def lq(rb=g + 2):
                        x8q[rb] = load_x8(rb, "x8q")
                    fifo.append(lq)
                elif jp == 2 and g + 4 < NG:
                    fifo.append(
                        lambda rb=g + 4: p1_qk8(rb, x8kv[rb], wk8_sb, kt_sb,
                                                None))
                elif jp == 3 and g + 4 < NG:
                    fifo.append(lambda rb=g + 4: p1_v(rb, xkv[rb], 0))
                    fifo.append(lambda rb=g + 4: p1_v(rb, xkv[rb], 1))
                elif jp == 4 and g + 4 < NG:
                    fifo.append(lambda rb=g + 4: p1_v(rb, xkv[rb], 2))
                    fifo.append(lambda rb=g + 4: p1_v(rb, xkv[rb], 3))
                elif jp == 5 and g + 5 < NG:
                    def lkv(rb=g + 5):
                        x8kv[rb] = load_x8(rb, "x8kv")
                        xkv[rb] = load_x(rb, "xtkv")
                    fifo.append(lkv)
                elif jp == 4 and g == 15:
                    fifo.append(lambda: p3_tile(44))
                elif jp == 5 and g == 15:
                    # 44/45 early enough that their VectorE casts drain
                    # before the final normalize's m copies; 46/47 are
                    # reserved for normalize_last itself
                    fifo.append(lambda: p3_tile(45))
                elif jp == 6 and g >= 4 and g != 15:
                    for tt in range(4 * (g - 4), 4 * (g - 4) + 2):
                        fifo.append(lambda tt=tt: p3_tile(tt))
                elif jp == 7 and g >= 4 and g != 15:
                    for tt in range(4 * (g - 4) + 2, 4 * (g - 4) + 4):
                        fifo.append(lambda tt=tt: p3_tile(tt))
                if g >= 13 and 3 <= jp <= 6:
                    # last batch: also drip the previous block's p3 tiles
                    tt = 4 * (g - 1) + (jp - 3)
                    fifo.append(lambda tt=tt: p3_tile(tt))

            # ---- the flat pipeline. One fill point per step (chunky chains
            # minimize PE semaphore-check hops, ~90ns per chain switch) ----
            for t in range(NSTEP + 2):
                if t < NSTEP:
                    g, jp = divmod(t, NJP)
                    enqueue(g, jp)
                    emit_st(g, jp)
                if t >= 2:
                    pg, pjp = divmod(t - 2, NJP)
                    emit_pv(pg, pjp)
                fill(3 if t < 4 else 2)
                if t >= 2:
                    pg, pjp = divmod(t - 2, NJP)
                    if pjp == NJP - 1:
                        if pg == NG - 1:
                            normalize_last(pg)
                        else:
                            normalize(pg)
            # tail: drain leftovers + output projection of the last q-block
            fill(len(fifo))
            for tt in range(60, 64):
                p3_tile_tail(tt)
    return nc


_NC_CACHE = None


def _get_nc():
    global _NC_CACHE
    if _NC_CACHE is None:
        _NC_CACHE = build_bass()
    return _NC_CACHE


def make_in_maps(x, Wq, bq, Wk, bk, Wv, bv, Wo, bo):
    xt = np.ascontiguousarray(
        np.asarray(x, dtype=np.float32).reshape(R, EMB).astype(bf16).T
    )
    xt8 = np.ascontiguousarray(
        np.asarray(x, dtype=np.float32).reshape(R, EMB).astype(f8e4).T
    )

    def pack_w8(W, rows):
        # [p, (j, i, m)] with emb row c = j*256 + i*128 + p; 16x pre-scale
        Wc = np.ascontiguousarray((np.asarray(W)[rows, :] * 16.0).T)  # [c, m]
        arr = Wc.reshape(4, 2, 128, F).transpose(2, 0, 1, 3)  # [p, j, i, m]
        return np.ascontiguousarray(arr.reshape(128, 8 * F).astype(f8e4))
    in_maps = []
    for c in range(NCORES):
        rows = slice(F * c, F * (c + 1))
        in_maps.append({
            "xt": xt,
            "xt8": xt8,
            "wq8": pack_w8(Wq, rows),
            "wk8": pack_w8(Wk, rows),
            "wv": np.ascontiguousarray(np.asarray(Wv)[rows, :].T.astype(bf16)),
            "wo": np.ascontiguousarray(np.asarray(Wo)[:, rows].T.astype(bf16)),
            "bq": (np.asarray(bq)[rows] * 16.0).reshape(F, 1)
            .astype(np.float32),
            "bv": np.asarray(bv)[rows].reshape(1, F).astype(np.float32),
        })
    return in_maps


def gather(results, bo):
    acc = np.zeros((R, EMB), np.float32)
    for r in results:
        acc += r["out"].astype(np.float32)
    acc += np.asarray(bo, dtype=np.float32)
    return acc.reshape(B, T, EMB)


def kernel(x, Wq, bq, Wk, bk, Wv, bv, Wo, bo, _trace=False):
    nc = _get_nc()
    in_maps = make_in_maps(x, Wq, bq, Wk, bk, Wv, bv, Wo, bo)
    res = run_bass_kernel_spmd(nc, in_maps, list(range(NCORES)), trace=_trace)
    out = gather(res.results, bo)
    if _trace:
        kernel.last_result = res
    return out
